# revision 1
# baseline (speedup 1.0000x reference)
"""Trainium2 Bass kernel for nn_Attention_850403524681.

Windowed attention block: LayerNorm -> FiLM (cond) -> QKV -> per-head
RMS-norm(q,k) -> attention with rel-pos bias -> out projection.

Full shapes: x (512, 65, 1024) f32, cond (512, 1024) f32.
Sharding: data-parallel over the 512 window-batch dim across 8 cores
(64 windows per core); all parameters replicated.

Per-core dataflow (B=64 windows, T=65 tokens, NT=4160, D=1024):
  token-major LN (stats per token) -> xn -> DRAM -> DMA-xbar-transpose
  -> d-major xnT -> FiLM applied with window-broadcast APs -> x_fT
  -> QKV matmuls (token-major out) -> RMS-norm q/k token-major
  -> DRAM -> DMA-transpose -> feature-major qT/kT (gamma folded into kT)
  -> per-window attention: simT = kT.T@qT (4x row-tiled PE), exp on ACT,
     rel-pos bias as exp(bias) multiply, attnV via PE with ones-column
     augmented V giving softmax denominators, per-partition normalize
  -> attn_out -> DRAM -> DMA-transpose -> out projection.
"""

import sys

if "/opt/trn_rl_repo" not in sys.path:
    sys.path.insert(0, "/opt/trn_rl_repo")

import numpy as np
from contextlib import ExitStack

import concourse.bass as bass
import concourse.mybir as mybir
import concourse.tile as tile
import concourse.bacc as bacc

import ml_dtypes

BF16 = ml_dtypes.bfloat16

DIM = 1024
HEADS = 32
DIM_HEAD = 32
WINDOW = 8
NUM_REG = 1
NUM_REL = (2 * WINDOW - 1) ** 2  # 225
T = WINDOW * WINDOW + NUM_REG  # 65 tokens per window
N_CORES = 8

F32 = mybir.dt.float32
BF = mybir.dt.bfloat16
AF = mybir.ActivationFunctionType
ALU = mybir.AluOpType


def _rel_pos_indices():
    pos = np.arange(WINDOW)
    gi, gj = np.meshgrid(pos, pos, indexing="ij")
    grid = np.stack([gi, gj], axis=-1).reshape(-1, 2)
    rel = grid[:, None, :] - grid[None, :, :] + (WINDOW - 1)
    idx = rel[..., 0] * (2 * WINDOW - 1) + rel[..., 1]
    out = np.full((T, T), NUM_REL, dtype=np.int32)
    out[NUM_REG:, NUM_REG:] = idx
    return out


REL_IDX = _rel_pos_indices()

# head-within-half (hh) -> exp_sb column-block j: the sim PSUM evacuation
# AP iterates (bank, slot) bank-major while head hh sits at bank hh%4,
# slot hh//4; j(hh) is the 4x4 transpose permutation (self-inverse).
def _blk(hh):
    return (hh % 4) * 4 + hh // 4


def _bc(ap, n):
    """Append a broadcast (stride 0) innermost free dim of size n."""
    return bass.AP(ap.tensor, ap.offset, ap.ap + [[0, n]])


def build_program(B, debug_taps=False):
    """Build the per-core Bass program for B windows (B*65 % 16 == 0)."""
    NT = B * T
    assert NT % 16 == 0, "DMA transpose needs row counts divisible by 16"
    n_tok_tiles = (NT + 127) // 128

    nc = bacc.Bacc("TRN2", target_bir_lowering=False, debug=False)

    x_in = nc.dram_tensor("x", [B, T, DIM], F32, kind="ExternalInput").ap()
    condT = nc.dram_tensor("condT", [DIM, B], BF, kind="ExternalInput").ap()
    fw1 = nc.dram_tensor("fw1", [DIM, 2 * DIM], BF, kind="ExternalInput").ap()
    fb1 = nc.dram_tensor("fb1", [2 * DIM], F32, kind="ExternalInput").ap()
    fw2 = nc.dram_tensor("fw2", [2 * DIM, 2 * DIM], BF, kind="ExternalInput").ap()
    fb2 = nc.dram_tensor("fb2", [2 * DIM], F32, kind="ExternalInput").ap()
    wqk = nc.dram_tensor("wqk", [DIM, 2 * DIM], BF, kind="ExternalInput").ap()
    wv = nc.dram_tensor("wv", [DIM, DIM], BF, kind="ExternalInput").ap()
    wout = nc.dram_tensor("wout", [DIM, DIM], BF, kind="ExternalInput").ap()
    gcomb = nc.dram_tensor("gcomb", [DIM], F32, kind="ExternalInput").ap()
    expbT = nc.dram_tensor("expbT", [T, 2 * 16 * T], BF, kind="ExternalInput").ap()
    out_d = nc.dram_tensor("out", [B, T, DIM], F32, kind="ExternalOutput").ap()

    dbg = dict(kind="ExternalOutput") if debug_taps else {}
    xn_d = nc.dram_tensor("xn_d", [NT, DIM], BF, **dbg).ap()
    qk_d = nc.dram_tensor("qk_d", [NT, 2 * DIM], BF, **dbg).ap()
    v_d = nc.dram_tensor("v_d", [NT, HEADS * 33], BF, **dbg).ap()
    attn_d = nc.dram_tensor("attn_d", [NT, DIM], BF, **dbg).ap()
    gb_dbg = nc.dram_tensor("gb_dbg", [128, 16, B], BF, **dbg).ap() if debug_taps else None
    xfT_dbg = nc.dram_tensor("xfT_dbg", [8, 128, NT], BF, **dbg).ap() if debug_taps else None
    xnT_dbg = nc.dram_tensor("xnT_dbg", [8, 128, NT], BF, **dbg).ap() if debug_taps else None

    x_flat = x_in.rearrange("b t d -> (b t) d")
    out_flat = out_d.rearrange("b t d -> (b t) d")

    with tile.TileContext(nc) as tc, ExitStack() as top:
        consts = top.enter_context(tc.tile_pool(name="consts", bufs=1))

        # ---- constant loads -------------------------------------------------
        wqk_sb = consts.tile([128, 8, 2 * DIM], BF)
        nc.sync.dma_start(out=wqk_sb, in_=wqk.rearrange("(c p) j -> p c j", p=128))
        wv_sb = consts.tile([128, 8, DIM], BF)
        nc.sync.dma_start(out=wv_sb, in_=wv.rearrange("(c p) j -> p c j", p=128))
        wout_sb = consts.tile([128, 8, DIM], BF)
        nc.sync.dma_start(out=wout_sb, in_=wout.rearrange("(c p) j -> p c j", p=128))
        gcomb_sb = consts.tile([128, 8], F32)
        nc.sync.dma_start(out=gcomb_sb, in_=gcomb.rearrange("(c p) -> p c", p=128))
        expb_sb = consts.tile([128, 2 * 16 * T], BF)
        nc.sync.dma_start(out=expb_sb[:T, :], in_=expbT)
        gbT_sb = consts.tile([128, 16, B], BF)  # chunks 0-7 gammaT, 8-15 betaT
        eps_ln = consts.tile([128, 1], F32)
        nc.vector.memset(eps_ln, 1e-5)
        eps_rms = consts.tile([128, 1], F32)
        nc.vector.memset(eps_rms, 1e-24)

        # ---- phase 1: FiLM params (feature-major) ---------------------------
        with tc.tile_pool(name="film", bufs=1) as filmp, \
             tc.tile_pool(name="filmps", bufs=4, space="PSUM") as filmps:
            fw1_sb = filmp.tile([128, 8, 2 * DIM], BF)
            nc.sync.dma_start(out=fw1_sb, in_=fw1.rearrange("(c p) j -> p c j", p=128))
            fw2_sb = filmp.tile([128, 16, 2 * DIM], BF)
            nc.sync.dma_start(out=fw2_sb, in_=fw2.rearrange("(c p) j -> p c j", p=128))
            fb1_sb = filmp.tile([128, 16], F32)
            nc.sync.dma_start(out=fb1_sb, in_=fb1.rearrange("(c p) -> p c", p=128))
            fb2_sb = filmp.tile([128, 16], F32)
            nc.sync.dma_start(out=fb2_sb, in_=fb2.rearrange("(c p) -> p c", p=128))
            condT_sb = filmp.tile([128, 8, B], BF)
            nc.sync.dma_start(out=condT_sb, in_=condT.rearrange("(c p) b -> p c b", p=128))
            hT_sb = filmp.tile([128, 16, B], BF)

            for c in range(16):
                ps = filmps.tile([128, B], F32, tag="fps")
                for k in range(8):
                    nc.tensor.matmul(
                        ps, lhsT=fw1_sb[:, k, c * 128:(c + 1) * 128],
                        rhs=condT_sb[:, k, :], start=(k == 0), stop=(k == 7))
                lin = filmp.tile([128, B], F32, tag="lin", name=f"lin{c}")
                nc.scalar.activation(lin, ps, AF.Identity,
                                     bias=fb1_sb[:, c:c + 1], scale=1.0)
                sg = filmp.tile([128, B], F32, tag="sg", name=f"sg{c}")
                nc.scalar.activation(sg, ps, AF.Sigmoid,
                                     bias=fb1_sb[:, c:c + 1], scale=1.0)
                nc.vector.tensor_mul(hT_sb[:, c, :], lin, sg)
            for c in range(16):
                ps = filmps.tile([128, B], F32, tag="fps")
                for k in range(16):
                    nc.tensor.matmul(
                        ps, lhsT=fw2_sb[:, k, c * 128:(c + 1) * 128],
                        rhs=hT_sb[:, k, :], start=(k == 0), stop=(k == 15))
                nc.scalar.activation(gbT_sb[:, c, :], ps, AF.Identity,
                                     bias=fb2_sb[:, c:c + 1], scale=1.0)
            if debug_taps:
                nc.sync.dma_start(out=gb_dbg, in_=gbT_sb)

        # ---- phases 2-4: LN -> xnT -> FiLM-T -> QKV -> RMS-norm -------------
        with tc.tile_pool(name="ln", bufs=3) as lnp, \
             tc.tile_pool(name="lns", bufs=4) as lns, \
             tc.tile_pool(name="xfT", bufs=1) as xfTp, \
             tc.tile_pool(name="qkvev", bufs=2) as qkvev, \
             tc.tile_pool(name="qkvps", bufs=7, space="PSUM") as qkvps:

            # LN token-major -> xn_d
            for t in range(n_tok_tiles):
                r0, r1 = t * 128, min(t * 128 + 128, NT)
                R = r1 - r0
                x_t = lnp.tile([128, DIM], F32, tag="x")
                nc.sync.dma_start(out=x_t[:R], in_=x_flat[r0:r1])
                stats = lns.tile([128, 2, 6], F32, tag="st")
                for sg in range(2):
                    nc.vector.bn_stats(stats[:R, sg], x_t[:R, sg * 512:(sg + 1) * 512])
                mv = lns.tile([128, 2], F32, tag="mv")
                nc.vector.bn_aggr(mv[:R], stats[:R])
                rstd = lns.tile([128, 1], F32, tag="rs")
                nc.scalar.activation(rstd[:R], mv[:R, 1:2], AF.Sqrt, bias=eps_ln[:R])
                nc.vector.reciprocal(rstd[:R], rstd[:R])
                nm = lns.tile([128, 1], F32, tag="nm")
                nc.vector.tensor_scalar(nm[:R], mv[:R, 0:1], rstd[:R], -1.0,
                                        ALU.mult, ALU.mult)
                xn_t = lnp.tile([128, DIM], BF, tag="xn")
                nc.scalar.activation(xn_t[:R], x_t[:R], AF.Identity,
                                     bias=nm[:R], scale=rstd[:R])
                nc.sync.dma_start(out=xn_d[r0:r1], in_=xn_t[:R])

            # xnT via DMA transpose + FiLM in transposed domain
            x_fT = []
            for c in range(8):
                xnT_c = lnp.tile([128, NT], BF, tag="xnT")
                eng = nc.sync  # transposes must share one HWDGE ring:
                # concurrent xbar transposes on both rings corrupt each other
                eng.dma_start_transpose(out=xnT_c, in_=xn_d[:, c * 128:(c + 1) * 128])
                if debug_taps:
                    nc.sync.dma_start(out=xnT_dbg[c], in_=xnT_c)
                xf_c = xfTp.tile([128, NT], BF, tag=f"xfT{c}")
                xnv = xnT_c[:, :].rearrange("p (b t) -> p b t", b=B)
                xfv = xf_c[:, :].rearrange("p (b t) -> p b t", b=B)
                nc.vector.tensor_tensor(xfv, xnv, _bc(gbT_sb[:, c, :], T), ALU.mult)
                nc.vector.tensor_tensor(xfv, xfv, _bc(gbT_sb[:, 8 + c, :], T), ALU.add)
                if debug_taps:
                    nc.sync.dma_start(out=xfT_dbg[c], in_=xf_c)
                x_fT.append(xf_c)

            # QKV matmuls (token-major) + RMS-norm + stores
            for t in range(n_tok_tiles):
                r0, r1 = t * 128, min(t * 128 + 128, NT)
                R = r1 - r0
                ps = [qkvps.tile([128, 512], F32, tag="qp", name=f"qp{i}") for i in range(6)]
                for k in range(8):
                    lhsT = x_fT[k][:, r0:r1]
                    for n in range(6):
                        rhs = (wqk_sb[:, k, n * 512:(n + 1) * 512] if n < 4
                               else wv_sb[:, k, (n - 4) * 512:(n - 3) * 512])
                        nc.tensor.matmul(ps[n][:R], lhsT=lhsT, rhs=rhs,
                                         start=(k == 0), stop=(k == 7))
                q_t = qkvev.tile([128, 2 * DIM], BF, tag="qt")
                for n in range(4):
                    nc.scalar.copy(q_t[:R, n * 512:(n + 1) * 512], ps[n][:R])
                q2 = qkvev.tile([128, 2 * DIM], BF, tag="q2")
                nc.vector.tensor_mul(q2[:R], q_t[:R], q_t[:R])
                ss = lns.tile([128, 64], F32, tag="ss")
                nc.vector.tensor_reduce(
                    ss[:R], q2[:R, :].rearrange("p (h d) -> p h d", d=DIM_HEAD),
                    axis=mybir.AxisListType.X, op=ALU.add)
                nc.scalar.activation(ss[:R], ss[:R], AF.Sqrt, bias=eps_rms[:R])
                nc.vector.reciprocal(ss[:R], ss[:R])
                qv = q_t[:R, :].rearrange("p (h d) -> p h d", d=DIM_HEAD)
                nc.vector.tensor_tensor(qv, qv, _bc(ss[:R, :], DIM_HEAD), ALU.mult)
                nc.sync.dma_start(out=qk_d[r0:r1], in_=q_t[:R])
                v_t = qkvev.tile([128, HEADS, 33], BF, tag="vt")
                nc.vector.memset(v_t[:, :, 32:33], 1.0)
                for n in range(2):
                    nc.scalar.copy(
                        v_t[:R, n * 16:(n + 1) * 16, 0:32], ps[4 + n][:R])
                nc.sync.dma_start(out=v_d[r0:r1], in_=v_t[:R, :, :])

        # ---- phases 5-6: qT/kT + attention ----------------------------------
        with tc.tile_pool(name="qkT", bufs=1) as qkTp, \
             tc.tile_pool(name="att", bufs=2) as attp, \
             tc.tile_pool(name="atts", bufs=4) as atts, \
             tc.tile_pool(name="attps", bufs=2, space="PSUM") as attps:

            # process windows in groups so the resident qT/kT stays small;
            # group row counts must stay multiples of 16 for DMA transpose
            n_wg = 2 if (B % 32 == 0) else 1
            BG = B // n_wg
            for wg in range(n_wg):
              g0 = wg * BG * T
              qkT = []
              for c in range(16):
                qkT_c = qkTp.tile([128, BG * T], BF, tag=f"qkT{c}",
                                  name=f"qkT{wg}_{c}")
                eng = nc.sync  # transposes must share one HWDGE ring:
                # concurrent xbar transposes on both rings corrupt each other
                eng.dma_start_transpose(
                    out=qkT_c, in_=qk_d[g0:g0 + BG * T, c * 128:(c + 1) * 128])
                if c >= 8:  # kT: fold 32 * q_gamma * k_gamma per feature
                    nc.vector.tensor_scalar_mul(qkT_c, qkT_c, gcomb_sb[:, c - 8:c - 7])
                qkT.append(qkT_c)

              for bw in range(BG):
                b = wg * BG + bw
                c0, c1 = b * T, (b + 1) * T
                w0, w1 = bw * T, (bw + 1) * T
                v_b = attp.tile([128, HEADS * 33], BF, tag="vb")
                nc.sync.dma_start(out=v_b[:T], in_=v_d[c0:c1])
                for hf in range(2):
                    sim = attps.tile([128, 2048], F32, tag="ap")
                    for hh in range(16):
                        h = hf * 16 + hh
                        ch, rb = h // 4, 32 * (h % 4)
                        col = 512 * (hh % 4) + T * (hh // 4)
                        nc.tensor.matmul(
                            sim[0:T, col:col + T],
                            lhsT=qkT[8 + ch][rb:rb + 32, w0:w1],
                            rhs=qkT[ch][rb:rb + 32, w0:w1],
                            start=True, stop=True, tile_position=(rb, 0))
                    exp_h = attp.tile([128, 16 * T], BF, tag="exp")
                    simv = sim[0:T, :]
                    sim_ap = bass.AP(simv.tensor, simv.offset,
                                     [simv.ap[0], [512, 4], [T, 4], [1, T]])
                    nc.scalar.activation(
                        exp_h[0:T, :].rearrange("p (a s q) -> p a s q", a=4, s=4),
                        sim_ap, AF.Exp)
                    nc.vector.tensor_mul(
                        exp_h[0:T], exp_h[0:T],
                        expb_sb[0:T, hf * 16 * T:(hf + 1) * 16 * T])
                    out2 = attps.tile([128, 2048], F32, tag="ap")
                    for hh in range(16):
                        h = hf * 16 + hh
                        j = _blk(hh)
                        oc = 512 * (hh // 8) + 33 * (hh % 8)
                        nc.tensor.matmul(
                            out2[0:T, oc:oc + 33],
                            lhsT=exp_h[0:T, j * T:(j + 1) * T],
                            rhs=v_b[0:T, h * 33:(h + 1) * 33],
                            start=True, stop=True)
                    rs = atts.tile([128, 16], F32, tag="rs")
                    sv = out2[0:T, 32:33]
                    nc.vector.reciprocal(
                        rs[0:T, :].rearrange("p (a h) -> p a h", a=2),
                        bass.AP(sv.tensor, sv.offset, [sv.ap[0], [512, 2], [33, 8]]))
                    a_sb = attp.tile([128, 512], BF, tag="asb")
                    o2 = out2[0:T, :]
                    in0 = bass.AP(o2.tensor, o2.offset,
                                  [o2.ap[0], [512, 2], [33, 8], [1, 32]])
                    rsv = rs[0:T, :].rearrange("p (a h) -> p a h", a=2)
                    nc.vector.tensor_tensor(
                        a_sb[0:T, :].rearrange("p (a h d) -> p a h d", a=2, h=8),
                        in0, _bc(rsv, 32), ALU.mult)
                    nc.sync.dma_start(
                        out=attn_d[c0:c1, hf * 512:(hf + 1) * 512], in_=a_sb[0:T])

        # ---- phases 7-8: attn_outT + out projection -------------------------
        with tc.tile_pool(name="aT", bufs=1) as aTp, \
             tc.tile_pool(name="oev", bufs=3) as oev, \
             tc.tile_pool(name="ops", bufs=4, space="PSUM") as ops:
            aT = []
            for c in range(8):
                aT_c = aTp.tile([128, NT], BF, tag=f"aT{c}")
                eng = nc.sync  # transposes must share one HWDGE ring:
                # concurrent xbar transposes on both rings corrupt each other
                eng.dma_start_transpose(out=aT_c, in_=attn_d[:, c * 128:(c + 1) * 128])
                aT.append(aT_c)
            for t in range(n_tok_tiles):
                r0, r1 = t * 128, min(t * 128 + 128, NT)
                R = r1 - r0
                ps = [ops.tile([128, 512], F32, tag="op", name=f"op{i}") for i in range(2)]
                for k in range(8):
                    for n in range(2):
                        nc.tensor.matmul(
                            ps[n][:R], lhsT=aT[k][:, r0:r1],
                            rhs=wout_sb[:, k, n * 512:(n + 1) * 512],
                            start=(k == 0), stop=(k == 7))
                o_t = oev.tile([128, DIM], F32, tag="ot")
                for n in range(2):
                    nc.scalar.copy(o_t[:R, n * 512:(n + 1) * 512], ps[n][:R])
                nc.sync.dma_start(out=out_flat[r0:r1], in_=o_t[:R])

    nc.finalize()
    return nc


def host_inputs(x, cond, film_w1, film_b1, film_w2, film_b2, w_qkv,
                q_gamma, k_gamma, rel_bias_table, w_out, n_cores=N_CORES):
    """Build the per-core input maps (host-side staging only)."""
    Bfull = x.shape[0]
    B = Bfull // n_cores
    bias = np.asarray(rel_bias_table, np.float32)[REL_IDX]  # (q, k, h)
    expb = np.exp(bias.astype(np.float64)).astype(np.float32)
    expbT = np.zeros((T, 2 * 16 * T), np.float32)
    for hf in range(2):
        for j in range(16):
            H = hf * 16 + _blk(j)
            expbT[:, (hf * 16 + j) * T:(hf * 16 + j + 1) * T] = expb[:, :, H].T
    gcomb = (32.0 * np.asarray(q_gamma, np.float32).reshape(HEADS, DIM_HEAD)
             * np.asarray(k_gamma, np.float32).reshape(HEADS, DIM_HEAD)).reshape(-1)
    shared = {
        "fw1": np.asarray(film_w1).astype(BF16),
        "fb1": np.asarray(film_b1, np.float32),
        "fw2": np.asarray(film_w2).astype(BF16),
        "fb2": np.asarray(film_b2, np.float32),
        "wqk": np.asarray(w_qkv[:, :2 * DIM]).astype(BF16),
        "wv": np.ascontiguousarray(w_qkv[:, 2 * DIM:]).astype(BF16),
        "wout": np.asarray(w_out).astype(BF16),
        "gcomb": gcomb.astype(np.float32),
        "expbT": expbT.astype(BF16),
    }
    in_maps = []
    for i in range(n_cores):
        m = dict(shared)
        m["x"] = np.ascontiguousarray(x[i * B:(i + 1) * B], np.float32)
        m["condT"] = np.ascontiguousarray(
            np.asarray(cond[i * B:(i + 1) * B], np.float32).T).astype(BF16)
        in_maps.append(m)
    return in_maps


_PROGRAM_CACHE = {}


def _get_program(B):
    if B not in _PROGRAM_CACHE:
        _PROGRAM_CACHE[B] = build_program(B)
    return _PROGRAM_CACHE[B]


def run(inputs, trace=False, tmpdir=None):
    from concourse.bass_utils import run_bass_kernel_spmd

    x = np.asarray(inputs["x"], np.float32)
    B = x.shape[0] // N_CORES
    nc = _get_program(B)
    in_maps = host_inputs(**inputs)
    res = run_bass_kernel_spmd(nc, in_maps, core_ids=list(range(N_CORES)),
                               trace=trace, tmpdir=tmpdir)
    out = np.concatenate([np.asarray(r["out"]) for r in res.results], axis=0)
    return out.astype(np.float32), res


def kernel(x, cond, film_w1, film_b1, film_w2, film_b2, w_qkv,
           q_gamma, k_gamma, rel_bias_table, w_out):
    out, _ = run(dict(
        x=x, cond=cond, film_w1=film_w1, film_b1=film_b1, film_w2=film_w2,
        film_b2=film_b2, w_qkv=w_qkv, q_gamma=q_gamma, k_gamma=k_gamma,
        rel_bias_table=rel_bias_table, w_out=w_out))
    return out


def run_timed(inputs, iters=10):
    """Execute on 8 cores with device-resident inputs; time execute-only.

    Returns (out_full, per_iter_seconds). Mirrors bass2jax.run_bass_via_pjrt
    but pre-places inputs on the device mesh so the timed region covers only
    the sharded NEFF execution (plus PJRT dispatch).
    """
    import jax
    import numpy as _np
    from jax.sharding import Mesh, PartitionSpec, NamedSharding
    from jax.experimental.shard_map import shard_map
    from concourse import bass2jax, mybir as _mybir
    import time as _time

    bass2jax.install_neuronx_cc_hook()
    x = np.asarray(inputs["x"], np.float32)
    B = x.shape[0] // N_CORES
    nc = _get_program(B)
    in_maps = host_inputs(**inputs)

    in_names, out_names, out_avals, zero_shapes = [], [], [], []
    for alloc in nc.m.functions[0].allocations:
        if not isinstance(alloc, _mybir.MemoryLocationSet):
            continue
        name = alloc.memorylocations[0].name
        if alloc.kind == "ExternalInput":
            if nc.partition_id_tensor is None or name != nc.partition_id_tensor.name:
                in_names.append(name)
        elif alloc.kind == "ExternalOutput":
            out_names.append(name)
            shape = tuple(alloc.tensor_shape)
            dtype = _mybir.dt.np(alloc.dtype)
            out_avals.append(jax.core.ShapedArray(shape, dtype))
            zero_shapes.append((shape, dtype))
    n_params = len(in_names)
    all_in_names = in_names + out_names
    if nc.partition_id_tensor is not None:
        all_in_names = all_in_names + [nc.partition_id_tensor.name]

    def _body(*args):
        operands = list(args)
        if nc.partition_id_tensor is not None:
            operands.append(bass2jax.partition_id_tensor())
        outs = bass2jax._bass_exec_p.bind(
            *operands,
            out_avals=tuple(out_avals),
            in_names=tuple(all_in_names),
            out_names=tuple(out_names),
            lowering_input_output_aliases=(),
            sim_require_finite=True,
            sim_require_nnan=True,
            nc=nc,
        )
        return tuple(outs)

    devices = jax.devices()[:N_CORES]
    mesh = Mesh(_np.asarray(devices), ("core",))
    n_outs = len(out_names)
    donate = tuple(range(n_params, n_params + n_outs))
    sharded = jax.jit(
        shard_map(_body, mesh=mesh,
                  in_specs=(PartitionSpec("core"),) * (n_params + n_outs),
                  out_specs=(PartitionSpec("core"),) * n_outs,
                  check_rep=False),
        donate_argnums=donate, keep_unused=True)

    shard = NamedSharding(mesh, PartitionSpec("core"))
    dev_in = [
        jax.device_put(_np.concatenate(
            [_np.asarray(in_maps[c][name]) for c in range(N_CORES)], axis=0), shard)
        for name in in_names
    ]
    def fresh_zeros():
        return [jax.device_put(
            _np.zeros((N_CORES * s[0], *s[1:]), d), shard) for s, d in zero_shapes]

    # warm-up (compiles)
    outs = sharded(*dev_in, *fresh_zeros())
    jax.block_until_ready(outs)

    zsets = [fresh_zeros() for _ in range(iters)]
    times = []
    for z in zsets:
        t0 = _time.perf_counter()
        outs = sharded(*dev_in, *z)
        jax.block_until_ready(outs)
        times.append(_time.perf_counter() - t0)

    oi = out_names.index("out")
    full = _np.asarray(outs[oi]).reshape(N_CORES * B, T, DIM).astype(_np.float32)
    return full, times


def run_chained(inputs, n_chain=8, iters=5):
    """Measure per-execution device time by chaining n_chain sequential
    executions of the NEFF inside one PJRT dispatch (the 'out' donation
    buffer threads a data dependency), then comparing against a 1-chain
    dispatch. Returns (out, times_1, times_n)."""
    import jax
    import numpy as _np
    from jax.sharding import Mesh, PartitionSpec, NamedSharding
    from jax.experimental.shard_map import shard_map
    from concourse import bass2jax, mybir as _mybir
    import time as _time

    bass2jax.install_neuronx_cc_hook()
    x = np.asarray(inputs["x"], np.float32)
    B = x.shape[0] // N_CORES
    nc = _get_program(B)
    in_maps = host_inputs(**inputs)

    in_names, out_names, out_avals, zero_shapes = [], [], [], []
    for alloc in nc.m.functions[0].allocations:
        if not isinstance(alloc, _mybir.MemoryLocationSet):
            continue
        name = alloc.memorylocations[0].name
        if alloc.kind == "ExternalInput":
            if nc.partition_id_tensor is None or name != nc.partition_id_tensor.name:
                in_names.append(name)
        elif alloc.kind == "ExternalOutput":
            out_names.append(name)
            shape = tuple(alloc.tensor_shape)
            dtype = _mybir.dt.np(alloc.dtype)
            out_avals.append(jax.core.ShapedArray(shape, dtype))
            zero_shapes.append((shape, dtype))
    n_params = len(in_names)
    all_in_names = in_names + out_names
    if nc.partition_id_tensor is not None:
        all_in_names = all_in_names + [nc.partition_id_tensor.name]
    oi = out_names.index("out")

    def _exec_once(ins, outbufs):
        operands = list(ins) + list(outbufs)
        if nc.partition_id_tensor is not None:
            operands.append(bass2jax.partition_id_tensor())
        return bass2jax._bass_exec_p.bind(
            *operands, out_avals=tuple(out_avals), in_names=tuple(all_in_names),
            out_names=tuple(out_names), lowering_input_output_aliases=(),
            sim_require_finite=True, sim_require_nnan=True, nc=nc)

    def make_body(n):
        def _body(*args):
            ins = args[:n_params]
            outbufs = list(args[n_params:])
            for _ in range(n):
                outs = _exec_once(ins, outbufs)
                outbufs = list(outs)
            return tuple(outbufs)
        return _body

    devices = jax.devices()[:N_CORES]
    mesh = Mesh(_np.asarray(devices), ("core",))
    n_outs = len(out_names)
    donate = tuple(range(n_params, n_params + n_outs))
    shard = NamedSharding(mesh, PartitionSpec("core"))
    fns = {}
    for n in (1, n_chain):
        fns[n] = jax.jit(
            shard_map(make_body(n), mesh=mesh,
                      in_specs=(PartitionSpec("core"),) * (n_params + n_outs),
                      out_specs=(PartitionSpec("core"),) * n_outs,
                      check_rep=False),
            donate_argnums=donate, keep_unused=True)

    dev_in = [
        jax.device_put(_np.concatenate(
            [_np.asarray(in_maps[c][name]) for c in range(N_CORES)], axis=0), shard)
        for name in in_names
    ]
    def fresh_zeros():
        return [jax.device_put(
            _np.zeros((N_CORES * s[0], *s[1:]), d), shard) for s, d in zero_shapes]

    results = {}
    out_final = None
    for n in (1, n_chain):
        outs = fns[n](*dev_in, *fresh_zeros())
        jax.block_until_ready(outs)
        ts = []
        for _ in range(iters):
            z = fresh_zeros()
            t0 = _time.perf_counter()
            outs = fns[n](*dev_in, *z)
            jax.block_until_ready(outs)
            ts.append(_time.perf_counter() - t0)
        results[n] = ts
        out_final = outs
    full = _np.asarray(out_final[oi]).reshape(N_CORES * B, T, DIM).astype(_np.float32)
    return full, results[1], results[n_chain]



# revision 2
# speedup vs baseline: 15055.1640x; 15055.1640x over previous
"""Trainium2 Bass kernel for nn_Attention_850403524681.

Windowed attention block: LayerNorm -> FiLM (cond) -> QKV -> per-head
RMS-norm(q,k) -> attention with rel-pos bias -> out projection.

Full shapes: x (512, 65, 1024) f32, cond (512, 1024) f32.
Sharding: data-parallel over the 512 window-batch dim across 8 cores
(64 windows per core); all parameters replicated.

Per-core dataflow (B=64 windows, T=65 tokens, NT=4160, D=1024):
  token-major LN (stats per token) -> xn -> DRAM -> DMA-xbar-transpose
  -> d-major xnT -> FiLM applied with window-broadcast APs -> x_fT
  -> QKV matmuls (token-major out) -> RMS-norm q/k token-major
  -> DRAM -> DMA-transpose -> feature-major qT/kT (gamma folded into kT)
  -> per-window attention: simT = kT.T@qT (4x row-tiled PE), exp on ACT,
     rel-pos bias as exp(bias) multiply, attnV via PE with ones-column
     augmented V giving softmax denominators, per-partition normalize
  -> attn_out -> DRAM -> DMA-transpose -> out projection.
"""

import sys

if "/opt/trn_rl_repo" not in sys.path:
    sys.path.insert(0, "/opt/trn_rl_repo")

import numpy as np
from contextlib import ExitStack

import concourse.bass as bass
import concourse.mybir as mybir
import concourse.tile as tile
import concourse.bacc as bacc

import ml_dtypes

BF16 = ml_dtypes.bfloat16

DIM = 1024
HEADS = 32
DIM_HEAD = 32
WINDOW = 8
NUM_REG = 1
NUM_REL = (2 * WINDOW - 1) ** 2  # 225
T = WINDOW * WINDOW + NUM_REG  # 65 tokens per window
N_CORES = 8

F32 = mybir.dt.float32
BF = mybir.dt.bfloat16
AF = mybir.ActivationFunctionType
ALU = mybir.AluOpType


def _rel_pos_indices():
    pos = np.arange(WINDOW)
    gi, gj = np.meshgrid(pos, pos, indexing="ij")
    grid = np.stack([gi, gj], axis=-1).reshape(-1, 2)
    rel = grid[:, None, :] - grid[None, :, :] + (WINDOW - 1)
    idx = rel[..., 0] * (2 * WINDOW - 1) + rel[..., 1]
    out = np.full((T, T), NUM_REL, dtype=np.int32)
    out[NUM_REG:, NUM_REG:] = idx
    return out


REL_IDX = _rel_pos_indices()

# head-within-half (hh) -> exp_sb column-block j: the sim PSUM evacuation
# AP iterates (bank, slot) bank-major while head hh sits at bank hh%4,
# slot hh//4; j(hh) is the 4x4 transpose permutation (self-inverse).
def _blk(hh):
    return (hh % 4) * 4 + hh // 4


def _bc(ap, n):
    """Append a broadcast (stride 0) innermost free dim of size n."""
    return bass.AP(ap.tensor, ap.offset, ap.ap + [[0, n]])


def build_program(B, debug_taps=False):
    """Build the per-core Bass program for B windows (B*65 % 16 == 0)."""
    NT = B * T
    assert NT % 16 == 0, "DMA transpose needs row counts divisible by 16"
    n_tok_tiles = (NT + 127) // 128

    nc = bacc.Bacc("TRN2", target_bir_lowering=False, debug=False)

    x_in = nc.dram_tensor("x", [B, T, DIM], F32, kind="ExternalInput").ap()
    condT = nc.dram_tensor("condT", [DIM, B], BF, kind="ExternalInput").ap()
    fw1 = nc.dram_tensor("fw1", [DIM, 2 * DIM], BF, kind="ExternalInput").ap()
    fb1 = nc.dram_tensor("fb1", [2 * DIM], F32, kind="ExternalInput").ap()
    fw2 = nc.dram_tensor("fw2", [2 * DIM, 2 * DIM], BF, kind="ExternalInput").ap()
    fb2 = nc.dram_tensor("fb2", [2 * DIM], F32, kind="ExternalInput").ap()
    wqk = nc.dram_tensor("wqk", [DIM, 2 * DIM], BF, kind="ExternalInput").ap()
    wv = nc.dram_tensor("wv", [DIM, DIM], BF, kind="ExternalInput").ap()
    wout = nc.dram_tensor("wout", [DIM, DIM], BF, kind="ExternalInput").ap()
    gcomb = nc.dram_tensor("gcomb", [DIM], F32, kind="ExternalInput").ap()
    expbT = nc.dram_tensor("expbT", [T, 2 * 16 * T], BF, kind="ExternalInput").ap()
    out_d = nc.dram_tensor("out", [B, T, DIM], F32, kind="ExternalOutput").ap()

    dbg = dict(kind="ExternalOutput") if debug_taps else {}
    xn_d = nc.dram_tensor("xn_d", [NT, DIM], BF, **dbg).ap()
    qk_d = nc.dram_tensor("qk_d", [NT, 2 * DIM], BF, **dbg).ap()
    v_d = nc.dram_tensor("v_d", [NT, HEADS * 33], BF, **dbg).ap()
    attn_d = nc.dram_tensor("attn_d", [NT, DIM], BF, **dbg).ap()
    gb_dbg = nc.dram_tensor("gb_dbg", [128, 16, B], BF, **dbg).ap() if debug_taps else None
    xfT_dbg = nc.dram_tensor("xfT_dbg", [8, 128, NT], BF, **dbg).ap() if debug_taps else None
    xnT_dbg = nc.dram_tensor("xnT_dbg", [8, 128, NT], BF, **dbg).ap() if debug_taps else None

    x_flat = x_in.rearrange("b t d -> (b t) d")
    out_flat = out_d.rearrange("b t d -> (b t) d")

    with tile.TileContext(nc) as tc, ExitStack() as top:
        consts = top.enter_context(tc.tile_pool(name="consts", bufs=1))

        # ---- constant loads -------------------------------------------------
        wqk_sb = consts.tile([128, 8, 2 * DIM], BF)
        nc.sync.dma_start(out=wqk_sb, in_=wqk.rearrange("(c p) j -> p c j", p=128))
        wv_sb = consts.tile([128, 8, DIM], BF)
        nc.sync.dma_start(out=wv_sb, in_=wv.rearrange("(c p) j -> p c j", p=128))
        wout_sb = consts.tile([128, 8, DIM], BF)
        nc.sync.dma_start(out=wout_sb, in_=wout.rearrange("(c p) j -> p c j", p=128))
        gcomb_sb = consts.tile([128, 8], F32)
        nc.sync.dma_start(out=gcomb_sb, in_=gcomb.rearrange("(c p) -> p c", p=128))
        expb_sb = consts.tile([128, 2 * 16 * T], BF)
        nc.sync.dma_start(out=expb_sb[:T, :], in_=expbT)
        gbT_sb = consts.tile([128, 16, B], BF)  # chunks 0-7 gammaT, 8-15 betaT
        eps_ln = consts.tile([128, 1], F32)
        nc.vector.memset(eps_ln, 1e-5)
        eps_rms = consts.tile([128, 1], F32)
        nc.vector.memset(eps_rms, 1e-24)

        # ---- phase 1: FiLM params (feature-major) ---------------------------
        with tc.tile_pool(name="film", bufs=1) as filmp, \
             tc.tile_pool(name="filmps", bufs=4, space="PSUM") as filmps:
            fw1_sb = filmp.tile([128, 8, 2 * DIM], BF)
            nc.sync.dma_start(out=fw1_sb, in_=fw1.rearrange("(c p) j -> p c j", p=128))
            fw2_sb = filmp.tile([128, 16, 2 * DIM], BF)
            nc.sync.dma_start(out=fw2_sb, in_=fw2.rearrange("(c p) j -> p c j", p=128))
            fb1_sb = filmp.tile([128, 16], F32)
            nc.sync.dma_start(out=fb1_sb, in_=fb1.rearrange("(c p) -> p c", p=128))
            fb2_sb = filmp.tile([128, 16], F32)
            nc.sync.dma_start(out=fb2_sb, in_=fb2.rearrange("(c p) -> p c", p=128))
            condT_sb = filmp.tile([128, 8, B], BF)
            nc.sync.dma_start(out=condT_sb, in_=condT.rearrange("(c p) b -> p c b", p=128))
            hT_sb = filmp.tile([128, 16, B], BF)

            for c in range(16):
                ps = filmps.tile([128, B], F32, tag="fps")
                for k in range(8):
                    nc.tensor.matmul(
                        ps, lhsT=fw1_sb[:, k, c * 128:(c + 1) * 128],
                        rhs=condT_sb[:, k, :], start=(k == 0), stop=(k == 7))
                lin = filmp.tile([128, B], F32, tag="lin", name=f"lin{c}")
                nc.scalar.activation(lin, ps, AF.Identity,
                                     bias=fb1_sb[:, c:c + 1], scale=1.0)
                sg = filmp.tile([128, B], F32, tag="sg", name=f"sg{c}")
                nc.scalar.activation(sg, ps, AF.Sigmoid,
                                     bias=fb1_sb[:, c:c + 1], scale=1.0)
                nc.vector.tensor_mul(hT_sb[:, c, :], lin, sg)
            for c in range(16):
                ps = filmps.tile([128, B], F32, tag="fps")
                for k in range(16):
                    nc.tensor.matmul(
                        ps, lhsT=fw2_sb[:, k, c * 128:(c + 1) * 128],
                        rhs=hT_sb[:, k, :], start=(k == 0), stop=(k == 15))
                nc.scalar.activation(gbT_sb[:, c, :], ps, AF.Identity,
                                     bias=fb2_sb[:, c:c + 1], scale=1.0)
            if debug_taps:
                nc.sync.dma_start(out=gb_dbg, in_=gbT_sb)

        # ---- phases 2-4: LN -> xnT -> FiLM-T -> QKV -> RMS-norm -------------
        with tc.tile_pool(name="ln", bufs=3) as lnp, \
             tc.tile_pool(name="lns", bufs=4) as lns, \
             tc.tile_pool(name="xfT", bufs=1) as xfTp, \
             tc.tile_pool(name="qkvev", bufs=2) as qkvev, \
             tc.tile_pool(name="qkvps", bufs=7, space="PSUM") as qkvps:

            # LN token-major -> xn_d
            for t in range(n_tok_tiles):
                r0, r1 = t * 128, min(t * 128 + 128, NT)
                R = r1 - r0
                x_t = lnp.tile([128, DIM], F32, tag="x")
                nc.sync.dma_start(out=x_t[:R], in_=x_flat[r0:r1])
                stats = lns.tile([128, 2, 6], F32, tag="st")
                for sg in range(2):
                    nc.vector.bn_stats(stats[:R, sg], x_t[:R, sg * 512:(sg + 1) * 512])
                mv = lns.tile([128, 2], F32, tag="mv")
                nc.vector.bn_aggr(mv[:R], stats[:R])
                rstd = lns.tile([128, 1], F32, tag="rs")
                nc.scalar.activation(rstd[:R], mv[:R, 1:2], AF.Sqrt, bias=eps_ln[:R])
                nc.vector.reciprocal(rstd[:R], rstd[:R])
                nm = lns.tile([128, 1], F32, tag="nm")
                nc.vector.tensor_scalar(nm[:R], mv[:R, 0:1], rstd[:R], -1.0,
                                        ALU.mult, ALU.mult)
                xn_t = lnp.tile([128, DIM], BF, tag="xn")
                nc.scalar.activation(xn_t[:R], x_t[:R], AF.Identity,
                                     bias=nm[:R], scale=rstd[:R])
                nc.sync.dma_start(out=xn_d[r0:r1], in_=xn_t[:R])

            # xnT via DMA transpose + FiLM in transposed domain
            x_fT = []
            for c in range(8):
                xnT_c = lnp.tile([128, NT], BF, tag="xnT")
                eng = nc.sync  # transposes must share one HWDGE ring:
                # concurrent xbar transposes on both rings corrupt each other
                eng.dma_start_transpose(out=xnT_c, in_=xn_d[:, c * 128:(c + 1) * 128])
                if debug_taps:
                    nc.sync.dma_start(out=xnT_dbg[c], in_=xnT_c)
                xf_c = xfTp.tile([128, NT], BF, tag=f"xfT{c}")
                xnv = xnT_c[:, :].rearrange("p (b t) -> p b t", b=B)
                xfv = xf_c[:, :].rearrange("p (b t) -> p b t", b=B)
                nc.vector.tensor_tensor(xfv, xnv, _bc(gbT_sb[:, c, :], T), ALU.mult)
                nc.vector.tensor_tensor(xfv, xfv, _bc(gbT_sb[:, 8 + c, :], T), ALU.add)
                if debug_taps:
                    nc.sync.dma_start(out=xfT_dbg[c], in_=xf_c)
                x_fT.append(xf_c)

            # QKV matmuls (token-major) + RMS-norm + stores
            for t in range(n_tok_tiles):
                r0, r1 = t * 128, min(t * 128 + 128, NT)
                R = r1 - r0
                ps = [qkvps.tile([128, 512], F32, tag="qp", name=f"qp{i}") for i in range(6)]
                for k in range(8):
                    lhsT = x_fT[k][:, r0:r1]
                    for n in range(6):
                        rhs = (wqk_sb[:, k, n * 512:(n + 1) * 512] if n < 4
                               else wv_sb[:, k, (n - 4) * 512:(n - 3) * 512])
                        nc.tensor.matmul(ps[n][:R], lhsT=lhsT, rhs=rhs,
                                         start=(k == 0), stop=(k == 7))
                q_t = qkvev.tile([128, 2 * DIM], BF, tag="qt")
                for n in range(4):
                    nc.scalar.copy(q_t[:R, n * 512:(n + 1) * 512], ps[n][:R])
                q2 = qkvev.tile([128, 2 * DIM], BF, tag="q2")
                nc.vector.tensor_mul(q2[:R], q_t[:R], q_t[:R])
                ss = lns.tile([128, 64], F32, tag="ss")
                nc.vector.tensor_reduce(
                    ss[:R], q2[:R, :].rearrange("p (h d) -> p h d", d=DIM_HEAD),
                    axis=mybir.AxisListType.X, op=ALU.add)
                nc.scalar.activation(ss[:R], ss[:R], AF.Sqrt, bias=eps_rms[:R])
                nc.vector.reciprocal(ss[:R], ss[:R])
                qv = q_t[:R, :].rearrange("p (h d) -> p h d", d=DIM_HEAD)
                nc.vector.tensor_tensor(qv, qv, _bc(ss[:R, :], DIM_HEAD), ALU.mult)
                nc.sync.dma_start(out=qk_d[r0:r1], in_=q_t[:R])
                v_t = qkvev.tile([128, HEADS, 33], BF, tag="vt")
                nc.vector.memset(v_t[:, :, 32:33], 1.0)
                for n in range(2):
                    nc.scalar.copy(
                        v_t[:R, n * 16:(n + 1) * 16, 0:32], ps[4 + n][:R])
                nc.sync.dma_start(out=v_d[r0:r1], in_=v_t[:R, :, :])

        # ---- phases 5-6: qT/kT + attention ----------------------------------
        with tc.tile_pool(name="qkT", bufs=1) as qkTp, \
             tc.tile_pool(name="att", bufs=2) as attp, \
             tc.tile_pool(name="atts", bufs=4) as atts, \
             tc.tile_pool(name="attps", bufs=2, space="PSUM") as attps:

            # process windows in groups so the resident qT/kT stays small;
            # group row counts must stay multiples of 16 for DMA transpose
            n_wg = 2 if (B % 32 == 0) else 1
            BG = B // n_wg
            for wg in range(n_wg):
              g0 = wg * BG * T
              qkT = []
              for c in range(16):
                qkT_c = qkTp.tile([128, BG * T], BF, tag=f"qkT{c}",
                                  name=f"qkT{wg}_{c}")
                eng = nc.sync  # transposes must share one HWDGE ring:
                # concurrent xbar transposes on both rings corrupt each other
                eng.dma_start_transpose(
                    out=qkT_c, in_=qk_d[g0:g0 + BG * T, c * 128:(c + 1) * 128])
                if c >= 8:  # kT: fold 32 * q_gamma * k_gamma per feature
                    nc.vector.tensor_scalar_mul(qkT_c, qkT_c, gcomb_sb[:, c - 8:c - 7])
                qkT.append(qkT_c)

              for bw in range(BG):
                b = wg * BG + bw
                c0, c1 = b * T, (b + 1) * T
                w0, w1 = bw * T, (bw + 1) * T
                v_b = attp.tile([128, HEADS * 33], BF, tag="vb")
                nc.sync.dma_start(out=v_b[:T], in_=v_d[c0:c1])
                for hf in range(2):
                    sim = attps.tile([128, 2048], F32, tag="ap")
                    for hh in range(16):
                        h = hf * 16 + hh
                        ch, rb = h // 4, 32 * (h % 4)
                        col = 512 * (hh % 4) + T * (hh // 4)
                        nc.tensor.matmul(
                            sim[0:T, col:col + T],
                            lhsT=qkT[8 + ch][rb:rb + 32, w0:w1],
                            rhs=qkT[ch][rb:rb + 32, w0:w1],
                            start=True, stop=True, tile_position=(rb, 0))
                    exp_h = attp.tile([128, 16 * T], BF, tag="exp")
                    simv = sim[0:T, :]
                    sim_ap = bass.AP(simv.tensor, simv.offset,
                                     [simv.ap[0], [512, 4], [T, 4], [1, T]])
                    nc.scalar.activation(
                        exp_h[0:T, :].rearrange("p (a s q) -> p a s q", a=4, s=4),
                        sim_ap, AF.Exp)
                    nc.vector.tensor_mul(
                        exp_h[0:T], exp_h[0:T],
                        expb_sb[0:T, hf * 16 * T:(hf + 1) * 16 * T])
                    out2 = attps.tile([128, 2048], F32, tag="ap")
                    for hh in range(16):
                        h = hf * 16 + hh
                        j = _blk(hh)
                        oc = 512 * (hh // 8) + 33 * (hh % 8)
                        nc.tensor.matmul(
                            out2[0:T, oc:oc + 33],
                            lhsT=exp_h[0:T, j * T:(j + 1) * T],
                            rhs=v_b[0:T, h * 33:(h + 1) * 33],
                            start=True, stop=True)
                    rs = atts.tile([128, 16], F32, tag="rs")
                    sv = out2[0:T, 32:33]
                    nc.vector.reciprocal(
                        rs[0:T, :].rearrange("p (a h) -> p a h", a=2),
                        bass.AP(sv.tensor, sv.offset, [sv.ap[0], [512, 2], [33, 8]]))
                    a_sb = attp.tile([128, 512], BF, tag="asb")
                    o2 = out2[0:T, :]
                    in0 = bass.AP(o2.tensor, o2.offset,
                                  [o2.ap[0], [512, 2], [33, 8], [1, 32]])
                    rsv = rs[0:T, :].rearrange("p (a h) -> p a h", a=2)
                    nc.vector.tensor_tensor(
                        a_sb[0:T, :].rearrange("p (a h d) -> p a h d", a=2, h=8),
                        in0, _bc(rsv, 32), ALU.mult)
                    nc.sync.dma_start(
                        out=attn_d[c0:c1, hf * 512:(hf + 1) * 512], in_=a_sb[0:T])

        # ---- phases 7-8: attn_outT + out projection -------------------------
        with tc.tile_pool(name="aT", bufs=1) as aTp, \
             tc.tile_pool(name="oev", bufs=3) as oev, \
             tc.tile_pool(name="ops", bufs=4, space="PSUM") as ops:
            aT = []
            for c in range(8):
                aT_c = aTp.tile([128, NT], BF, tag=f"aT{c}")
                eng = nc.sync  # transposes must share one HWDGE ring:
                # concurrent xbar transposes on both rings corrupt each other
                eng.dma_start_transpose(out=aT_c, in_=attn_d[:, c * 128:(c + 1) * 128])
                aT.append(aT_c)
            for t in range(n_tok_tiles):
                r0, r1 = t * 128, min(t * 128 + 128, NT)
                R = r1 - r0
                ps = [ops.tile([128, 512], F32, tag="op", name=f"op{i}") for i in range(2)]
                for k in range(8):
                    for n in range(2):
                        nc.tensor.matmul(
                            ps[n][:R], lhsT=aT[k][:, r0:r1],
                            rhs=wout_sb[:, k, n * 512:(n + 1) * 512],
                            start=(k == 0), stop=(k == 7))
                o_t = oev.tile([128, DIM], F32, tag="ot")
                for n in range(2):
                    nc.scalar.copy(o_t[:R, n * 512:(n + 1) * 512], ps[n][:R])
                nc.sync.dma_start(out=out_flat[r0:r1], in_=o_t[:R])

    nc.finalize()
    return nc


def host_inputs(x, cond, film_w1, film_b1, film_w2, film_b2, w_qkv,
                q_gamma, k_gamma, rel_bias_table, w_out, n_cores=N_CORES):
    """Build the per-core input maps (host-side staging only)."""
    Bfull = x.shape[0]
    B = Bfull // n_cores
    bias = np.asarray(rel_bias_table, np.float32)[REL_IDX]  # (q, k, h)
    expb = np.exp(bias.astype(np.float64)).astype(np.float32)
    expbT = np.zeros((T, 2 * 16 * T), np.float32)
    for hf in range(2):
        for j in range(16):
            H = hf * 16 + _blk(j)
            expbT[:, (hf * 16 + j) * T:(hf * 16 + j + 1) * T] = expb[:, :, H].T
    gcomb = (32.0 * np.asarray(q_gamma, np.float32).reshape(HEADS, DIM_HEAD)
             * np.asarray(k_gamma, np.float32).reshape(HEADS, DIM_HEAD)).reshape(-1)
    shared = {
        "fw1": np.asarray(film_w1).astype(BF16),
        "fb1": np.asarray(film_b1, np.float32),
        "fw2": np.asarray(film_w2).astype(BF16),
        "fb2": np.asarray(film_b2, np.float32),
        "wqk": np.asarray(w_qkv[:, :2 * DIM]).astype(BF16),
        "wv": np.ascontiguousarray(w_qkv[:, 2 * DIM:]).astype(BF16),
        "wout": np.asarray(w_out).astype(BF16),
        "gcomb": gcomb.astype(np.float32),
        "expbT": expbT.astype(BF16),
    }
    in_maps = []
    for i in range(n_cores):
        m = dict(shared)
        m["x"] = np.ascontiguousarray(x[i * B:(i + 1) * B], np.float32)
        m["condT"] = np.ascontiguousarray(
            np.asarray(cond[i * B:(i + 1) * B], np.float32).T).astype(BF16)
        in_maps.append(m)
    return in_maps


_PROGRAM_CACHE = {}


def _get_program(B):
    if B not in _PROGRAM_CACHE:
        _PROGRAM_CACHE[B] = build_program(B)
    return _PROGRAM_CACHE[B]


def run(inputs, trace=False, tmpdir=None):
    from concourse.bass_utils import run_bass_kernel_spmd

    x = np.asarray(inputs["x"], np.float32)
    B = x.shape[0] // N_CORES
    nc = _get_program(B)
    in_maps = host_inputs(**inputs)
    res = run_bass_kernel_spmd(nc, in_maps, core_ids=list(range(N_CORES)),
                               trace=trace, tmpdir=tmpdir)
    out = np.concatenate([np.asarray(r["out"]) for r in res.results], axis=0)
    return out.astype(np.float32), res


def kernel(x, cond, film_w1, film_b1, film_w2, film_b2, w_qkv,
           q_gamma, k_gamma, rel_bias_table, w_out):
    out, _ = run(dict(
        x=x, cond=cond, film_w1=film_w1, film_b1=film_b1, film_w2=film_w2,
        film_b2=film_b2, w_qkv=w_qkv, q_gamma=q_gamma, k_gamma=k_gamma,
        rel_bias_table=rel_bias_table, w_out=w_out))
    return out


def run_slope(inputs, n_lo=8, n_hi=72, rounds=5):
    """Measure per-execution device time by async-queuing chains of
    sequentially-dependent executions (output buffers threaded through
    donation) and taking the slope between an n_hi-chain and an n_lo-chain
    wall time: slope = (T_hi - T_lo) / (n_hi - n_lo).  The fixed dispatch /
    relay overhead (~90ms here) cancels; each link in the chain is a full
    kernel execution (all HBM traffic + compute), serialized by the data
    dependency.  Returns (out_full, per_exec_seconds_median, slopes)."""
    import time as _time
    import jax
    import numpy as _np
    from jax.sharding import Mesh, PartitionSpec, NamedSharding
    from jax.experimental.shard_map import shard_map
    from concourse import bass2jax, mybir as _mybir

    bass2jax.install_neuronx_cc_hook()
    x = np.asarray(inputs["x"], np.float32)
    B = x.shape[0] // N_CORES
    nc = _get_program(B)
    in_maps = host_inputs(**inputs)

    in_names, out_names, out_avals, zero_shapes = [], [], [], []
    for alloc in nc.m.functions[0].allocations:
        if not isinstance(alloc, _mybir.MemoryLocationSet):
            continue
        name = alloc.memorylocations[0].name
        if alloc.kind == "ExternalInput":
            if nc.partition_id_tensor is None or name != nc.partition_id_tensor.name:
                in_names.append(name)
        elif alloc.kind == "ExternalOutput":
            out_names.append(name)
            shape = tuple(alloc.tensor_shape)
            dtype = _mybir.dt.np(alloc.dtype)
            out_avals.append(jax.core.ShapedArray(shape, dtype))
            zero_shapes.append((shape, dtype))
    n_params = len(in_names)
    all_in_names = in_names + out_names
    if nc.partition_id_tensor is not None:
        all_in_names = all_in_names + [nc.partition_id_tensor.name]
    oi = out_names.index("out")

    def _body(*args):
        operands = list(args)
        if nc.partition_id_tensor is not None:
            operands.append(bass2jax.partition_id_tensor())
        outs = bass2jax._bass_exec_p.bind(
            *operands, out_avals=tuple(out_avals), in_names=tuple(all_in_names),
            out_names=tuple(out_names), lowering_input_output_aliases=(),
            sim_require_finite=True, sim_require_nnan=True, nc=nc)
        return tuple(outs)

    devices = jax.devices()[:N_CORES]
    mesh = Mesh(_np.asarray(devices), ("core",))
    n_outs = len(out_names)
    donate = tuple(range(n_params, n_params + n_outs))
    shard = NamedSharding(mesh, PartitionSpec("core"))
    fn = jax.jit(
        shard_map(_body, mesh=mesh,
                  in_specs=(PartitionSpec("core"),) * (n_params + n_outs),
                  out_specs=(PartitionSpec("core"),) * n_outs,
                  check_rep=False),
        donate_argnums=donate, keep_unused=True)

    dev_in = [
        jax.device_put(_np.concatenate(
            [_np.asarray(in_maps[c][name]) for c in range(N_CORES)], axis=0), shard)
        for name in in_names
    ]
    jax.block_until_ready(dev_in)

    outs = []
    for s, d in zero_shapes:
        f = jax.jit(lambda s=s, d=d: jax.numpy.zeros((N_CORES * s[0], *s[1:]), d),
                    out_shardings=shard)
        outs.append(f())
    outs = fn(*dev_in, *outs)  # warm-up (compiles the wrapper)
    jax.block_until_ready(outs)

    slopes = []
    for _ in range(rounds):
        ts = {}
        for n in (n_lo, n_hi):
            t0 = _time.perf_counter()
            for _i in range(n):
                outs = fn(*dev_in, *outs)
            jax.block_until_ready(outs)
            ts[n] = _time.perf_counter() - t0
        slopes.append((ts[n_hi] - ts[n_lo]) / (n_hi - n_lo))
    slopes.sort()
    per_exec = slopes[len(slopes) // 2]

    full = _np.asarray(outs[oi]).reshape(N_CORES * B, T, DIM).astype(_np.float32)
    return full, per_exec, slopes


def run_timed(inputs, iters=10):
    """Execute on 8 cores with device-resident inputs; time execute-only.

    Returns (out_full, per_iter_seconds). Mirrors bass2jax.run_bass_via_pjrt
    but pre-places inputs on the device mesh so the timed region covers only
    the sharded NEFF execution (plus PJRT dispatch).
    """
    import jax
    import numpy as _np
    from jax.sharding import Mesh, PartitionSpec, NamedSharding
    from jax.experimental.shard_map import shard_map
    from concourse import bass2jax, mybir as _mybir
    import time as _time

    bass2jax.install_neuronx_cc_hook()
    x = np.asarray(inputs["x"], np.float32)
    B = x.shape[0] // N_CORES
    nc = _get_program(B)
    in_maps = host_inputs(**inputs)

    in_names, out_names, out_avals, zero_shapes = [], [], [], []
    for alloc in nc.m.functions[0].allocations:
        if not isinstance(alloc, _mybir.MemoryLocationSet):
            continue
        name = alloc.memorylocations[0].name
        if alloc.kind == "ExternalInput":
            if nc.partition_id_tensor is None or name != nc.partition_id_tensor.name:
                in_names.append(name)
        elif alloc.kind == "ExternalOutput":
            out_names.append(name)
            shape = tuple(alloc.tensor_shape)
            dtype = _mybir.dt.np(alloc.dtype)
            out_avals.append(jax.core.ShapedArray(shape, dtype))
            zero_shapes.append((shape, dtype))
    n_params = len(in_names)
    all_in_names = in_names + out_names
    if nc.partition_id_tensor is not None:
        all_in_names = all_in_names + [nc.partition_id_tensor.name]

    def _body(*args):
        operands = list(args)
        if nc.partition_id_tensor is not None:
            operands.append(bass2jax.partition_id_tensor())
        outs = bass2jax._bass_exec_p.bind(
            *operands,
            out_avals=tuple(out_avals),
            in_names=tuple(all_in_names),
            out_names=tuple(out_names),
            lowering_input_output_aliases=(),
            sim_require_finite=True,
            sim_require_nnan=True,
            nc=nc,
        )
        return tuple(outs)

    devices = jax.devices()[:N_CORES]
    mesh = Mesh(_np.asarray(devices), ("core",))
    n_outs = len(out_names)
    donate = tuple(range(n_params, n_params + n_outs))
    sharded = jax.jit(
        shard_map(_body, mesh=mesh,
                  in_specs=(PartitionSpec("core"),) * (n_params + n_outs),
                  out_specs=(PartitionSpec("core"),) * n_outs,
                  check_rep=False),
        donate_argnums=donate, keep_unused=True)

    shard = NamedSharding(mesh, PartitionSpec("core"))
    dev_in = [
        jax.device_put(_np.concatenate(
            [_np.asarray(in_maps[c][name]) for c in range(N_CORES)], axis=0), shard)
        for name in in_names
    ]
    def fresh_zeros():
        return [jax.device_put(
            _np.zeros((N_CORES * s[0], *s[1:]), d), shard) for s, d in zero_shapes]

    # warm-up (compiles)
    outs = sharded(*dev_in, *fresh_zeros())
    jax.block_until_ready(outs)

    zsets = [fresh_zeros() for _ in range(iters)]
    times = []
    for z in zsets:
        t0 = _time.perf_counter()
        outs = sharded(*dev_in, *z)
        jax.block_until_ready(outs)
        times.append(_time.perf_counter() - t0)

    oi = out_names.index("out")
    full = _np.asarray(outs[oi]).reshape(N_CORES * B, T, DIM).astype(_np.float32)
    return full, times


def run_chained(inputs, n_chain=8, iters=5):
    """Measure per-execution device time by chaining n_chain sequential
    executions of the NEFF inside one PJRT dispatch (the 'out' donation
    buffer threads a data dependency), then comparing against a 1-chain
    dispatch. Returns (out, times_1, times_n)."""
    import jax
    import numpy as _np
    from jax.sharding import Mesh, PartitionSpec, NamedSharding
    from jax.experimental.shard_map import shard_map
    from concourse import bass2jax, mybir as _mybir
    import time as _time

    bass2jax.install_neuronx_cc_hook()
    x = np.asarray(inputs["x"], np.float32)
    B = x.shape[0] // N_CORES
    nc = _get_program(B)
    in_maps = host_inputs(**inputs)

    in_names, out_names, out_avals, zero_shapes = [], [], [], []
    for alloc in nc.m.functions[0].allocations:
        if not isinstance(alloc, _mybir.MemoryLocationSet):
            continue
        name = alloc.memorylocations[0].name
        if alloc.kind == "ExternalInput":
            if nc.partition_id_tensor is None or name != nc.partition_id_tensor.name:
                in_names.append(name)
        elif alloc.kind == "ExternalOutput":
            out_names.append(name)
            shape = tuple(alloc.tensor_shape)
            dtype = _mybir.dt.np(alloc.dtype)
            out_avals.append(jax.core.ShapedArray(shape, dtype))
            zero_shapes.append((shape, dtype))
    n_params = len(in_names)
    all_in_names = in_names + out_names
    if nc.partition_id_tensor is not None:
        all_in_names = all_in_names + [nc.partition_id_tensor.name]
    oi = out_names.index("out")

    def _exec_once(ins, outbufs):
        operands = list(ins) + list(outbufs)
        if nc.partition_id_tensor is not None:
            operands.append(bass2jax.partition_id_tensor())
        return bass2jax._bass_exec_p.bind(
            *operands, out_avals=tuple(out_avals), in_names=tuple(all_in_names),
            out_names=tuple(out_names), lowering_input_output_aliases=(),
            sim_require_finite=True, sim_require_nnan=True, nc=nc)

    def make_body(n):
        def _body(*args):
            ins = args[:n_params]
            outbufs = list(args[n_params:])
            for _ in range(n):
                outs = _exec_once(ins, outbufs)
                outbufs = list(outs)
            return tuple(outbufs)
        return _body

    devices = jax.devices()[:N_CORES]
    mesh = Mesh(_np.asarray(devices), ("core",))
    n_outs = len(out_names)
    donate = tuple(range(n_params, n_params + n_outs))
    shard = NamedSharding(mesh, PartitionSpec("core"))
    fns = {}
    for n in (1, n_chain):
        fns[n] = jax.jit(
            shard_map(make_body(n), mesh=mesh,
                      in_specs=(PartitionSpec("core"),) * (n_params + n_outs),
                      out_specs=(PartitionSpec("core"),) * n_outs,
                      check_rep=False),
            donate_argnums=donate, keep_unused=True)

    dev_in = [
        jax.device_put(_np.concatenate(
            [_np.asarray(in_maps[c][name]) for c in range(N_CORES)], axis=0), shard)
        for name in in_names
    ]
    def fresh_zeros():
        return [jax.device_put(
            _np.zeros((N_CORES * s[0], *s[1:]), d), shard) for s, d in zero_shapes]

    results = {}
    out_final = None
    for n in (1, n_chain):
        outs = fns[n](*dev_in, *fresh_zeros())
        jax.block_until_ready(outs)
        ts = []
        for _ in range(iters):
            z = fresh_zeros()
            t0 = _time.perf_counter()
            outs = fns[n](*dev_in, *z)
            jax.block_until_ready(outs)
            ts.append(_time.perf_counter() - t0)
        results[n] = ts
        out_final = outs
    full = _np.asarray(out_final[oi]).reshape(N_CORES * B, T, DIM).astype(_np.float32)
    return full, results[1], results[n_chain]

